# revision 56
# baseline (speedup 1.0000x reference)
"""GQA attention kernel for Trainium2, sharded over 8 NeuronCores.

Problem: X (1, 4096, 1024), H=16 q-heads, KVH=4 kv-heads, head_dim=64.
Sharding: 2 q-heads + their shared kv-head per core (tensor parallel over H).

v2 design notes (engine balance):
  - ScalarE (Act) exp over the 2x4096x4096 scores is the hard floor
    (~285us); everything else is scheduled to hide under it.
  - Scores for the two heads run CONCURRENTLY in disjoint PE row groups
    (K=64 contraction each, base partitions 0/64 -> tile_position auto).
  - attnV uses Pt q-slices as the stationary operand (FWL hides the
    128-col weight load) and streams V_aug (64 v-dims + ones col), so its
    moving-column count is ~65 per (head, ktile, qtile) instead of 1024.
    Output lands with q on partitions and the softmax denominator in
    column 64 -> per-partition reciprocal+mul normalization on DVE.
  - attnV(t-1) is issued after scores(t) so the PE never waits on the
    exp of the tile it just produced.
  - q/kv projections and the output projection are interleaved into the
    attention t-loops through a dedicated 1-bank aux PSUM pool so the
    Act engine starts early and never drains.

Layouts on device (per core):
  xt    : X^T             (1024 D, 4096 S) bf16 (host pre-transposed)
  qt    : Q^T             (128 = 2 heads x 64 d, 4096 q) bf16
  kvt   : [K^T; V^T]      (128 = 64 k-d + 64 v-d, 4096 s) bf16
  kt2   : K^T duplicated  (128 = two copies of 64 k-d, 4096 s) bf16
  v     : V natural+ones  (128 s-tile, 65) x 32 tiles bf16 (col 64 == 1.0)
  St    : scores^T        (128 k, 1024 q) f32 PSUM = K_tile.T @ Qt
  Pt    : exp(St/8)       (128 k, 1024 q) bf16 SBUF (ScalarE)
  ot    : Pt_slice.T @ V_aug  (128 q, 65) f32 PSUM accum per (h, qtile)
  otn   : ot / denom      (128 q, 32 qt, 128 ad) bf16 SBUF
  otT   : otn transposed  (128 ad, 128 q) bf16 (PE transpose per qtile)
  y     : partial output  (4096, 1024) bf16 = otT.T @ o_w[rows]
"""

import sys

import numpy as np

try:
    import concourse.bass as bass
except ImportError:  # grading env may not have concourse on sys.path
    for p in ("/opt/trn_rl_repo", "/root/.axon_site/_ro/trn_rl_repo"):
        if p not in sys.path:
            sys.path.append(p)
    import concourse.bass as bass

import bass_rust
import ml_dtypes
from concourse import mybir
from concourse.bass_utils import run_bass_kernel_spmd
from concourse.masks import make_identity
from concourse.tile import TileContext

BF16 = ml_dtypes.bfloat16

B, S, D = 1, 4096, 1024
H, KVH, HD = 16, 4, 64
NCORES = 8
HPC = H // NCORES          # 2 q heads per core
DQ = HPC * HD              # 128 projected q dims per core
DKV = 2 * HD               # 128 = k head + v head dims
QC = 1024                  # attention q-chunk
KT = 128                   # k tile (seq positions per score tile)
# bf16 Schraudolph exp for the DVE-offloaded softmax tiles:
# exp(s/8) ~= bitcast_bf16(int16(SCH_A*s + SCH_B)); C=8.5 calibrated
SCH_A = (2.0 ** 7) / np.log(2.0) / 8.0
SCH_B = 127.0 * 2.0 ** 7 - 8.5
NKT = S // KT              # 32
NQC = S // QC              # 4
NQT = S // 128             # 32 q-tiles of 128
NDC = D // 128             # 8 contraction chunks for projections
NDR = D // 256             # 4 fp8 DoubleRow chunks (2 k-subtiles each)
MM_N = 512                 # max matmul free dim (one PSUM bank, f32)
NJ = S // MM_N             # 8 projection column chunks

_COMPILED = None


def build_bass():
    nc = bass.Bass()
    fp32 = mybir.dt.float32
    bf16 = mybir.dt.bfloat16

    xt = nc.declare_dram_parameter("xt", [D, S], bf16, isOutput=False)
    qw = nc.declare_dram_parameter("qw", [D, DQ], bf16, isOutput=False)
    kvw = nc.declare_dram_parameter("kvw", [D, DKV], bf16, isOutput=False)
    ow = nc.declare_dram_parameter("ow", [DQ, D], bf16, isOutput=False)
    qb = nc.declare_dram_parameter("qb", [1, DQ], bf16, isOutput=False)
    kvb = nc.declare_dram_parameter("kvb", [1, DKV], bf16, isOutput=False)
    y = nc.declare_dram_parameter("y", [S, D], bf16, isOutput=True)

    with TileContext(nc) as tc:
        with (
            tc.tile_pool(name="singles", bufs=1) as singles,
            tc.tile_pool(name="pt_pool", bufs=10) as pt_pool,
            tc.tile_pool(name="bc_pool", bufs=4) as bc_pool,
            tc.tile_pool(name="ott", bufs=8) as ott_pool,
            tc.tile_pool(name="ysb", bufs=6) as ysb_pool,
            tc.tile_pool(name="ps_st", bufs=2, space="PSUM") as ps_st,
            tc.tile_pool(name="ps_ot", bufs=1, space="PSUM") as ps_ot,
            tc.tile_pool(name="ps_aux", bufs=1, space="PSUM") as ps_aux,
        ):
            # ---- constants / weights ----
            ident = singles.tile([128, 128], bf16)
            make_identity(nc, ident)

            qw_sb = singles.tile([128, NDC, DQ], bf16)
            nc.sync.dma_start(
                out=qw_sb, in_=qw[:, :].rearrange("(c p) m -> p c m", p=128)
            )
            kvw_sb = singles.tile([128, NDC, DKV], bf16)
            nc.sync.dma_start(
                out=kvw_sb, in_=kvw[:, :].rearrange("(c p) m -> p c m", p=128)
            )
            ow_sb = singles.tile([DQ, D], bf16)
            nc.sync.dma_start(out=ow_sb, in_=ow[:, :])
            qb_sb = singles.tile([1, DQ], bf16)
            nc.sync.dma_start(out=qb_sb, in_=qb[:, :])
            kvb_sb = singles.tile([1, DKV], bf16)
            nc.sync.dma_start(out=kvb_sb, in_=kvb[:, :])
            ones_row = singles.tile([1, MM_N], bf16)
            nc.vector.memset(ones_row, 1.0)

            # bulk X load rides the gpsimd DMA queue so the latency-critical
            # small DMAs (kt2 dup, y writes) on the sync queue aren't stuck
            # behind ~25us of queued xt traffic
            # chunk 0 rides the sync queue (it boots ~9us before gpsimd's),
            # landing the first projection input sooner
            xt_sb = singles.tile([128, NDC, S], bf16)
            for j in range(NJ):
                eng = nc.sync if j == 0 else nc.gpsimd
                eng.dma_start(
                    out=xt_sb[:, :, bass.ts(j, MM_N)],
                    in_=xt[:, :].rearrange("(c p) s -> p c s", p=128)[
                        :, :, bass.ts(j, MM_N)],
                )

            qt_sb = singles.tile([DQ, S], bf16)
            kvt_sb = singles.tile([DKV, S], bf16)
            kt2_sb = singles.tile([DKV, S], bf16)
            v_sb = singles.tile([128, NKT, HD + 1], bf16)
            nc.vector.memset(v_sb, 1.0)
            otn_sb = singles.tile([128, NQT, DQ], bf16)

            def proj_chunk_ops(j, kind, pool, tag):
                """Projection chunk as a list of fine-grained closures so the
                work can be drip-fed between attention iterations. Ops of one
                chunk must stay contiguous (single live tile per pool tag)."""
                sl = bass.ts(j, MM_N)
                w_sb, b_sb = (qw_sb, qb_sb) if kind == "q" else (kvw_sb, kvb_sb)
                box = {}

                def mk_mm(c):
                    def f():
                        if c == 0:
                            box["ps"] = pool.tile(
                                [128, MM_N], fp32, tag=tag, name="psp")
                        nc.tensor.matmul(
                            box["ps"], w_sb[:, c, :], xt_sb[:, c, sl],
                            start=(c == 0), stop=False,
                        )
                    return f

                def tail():
                    nc.tensor.matmul(
                        box["ps"], b_sb, ones_row, start=False, stop=True)
                    if kind == "q":
                        nc.vector.tensor_copy(qt_sb[:, sl], box["ps"])
                    else:
                        nc.vector.tensor_copy(kvt_sb[:, sl], box["ps"])
                        # duplicate K rows into both partition halves for
                        # the row-group-concurrent score matmuls
                        nc.sync.dma_start(
                            out=kt2_sb[0:HD, sl], in_=kvt_sb[0:HD, sl])
                        nc.sync.dma_start(
                            out=kt2_sb[HD:DKV, sl], in_=kvt_sb[0:HD, sl])

                return [mk_mm(c) for c in range(NDC)] + [tail]

            def emit_vtr(t, pool=None, tag=None):
                # V tile t into natural layout (s on partitions); col 64
                # stays at the memset 1.0 (softmax denominator trick)
                pool = pool if pool is not None else ps_aux
                tag = tag if tag is not None else "aux"
                pvt = pool.tile([128, HD], bf16, tag=tag, name="pvt")
                nc.tensor.transpose(
                    pvt, kvt_sb[HD:DKV, bass.ts(t, KT)], ident[HD:DKV, HD:DKV]
                )
                nc.vector.tensor_copy(v_sb[:, t, 0:HD], pvt)

            def ot_ap(ot_tile, h, qt):
                idx = h * (QC // 128) + qt
                bank, slot = idx // 6, idx % 6
                return ot_tile[:, bank, slot * (HD + 1):(slot + 1) * (HD + 1)]

            def emit_scores_exp(jc, t):
                sts = [
                    ps_st.tile([128, QC], fp32, tag="st", name=f"st{h}")
                    for h in range(HPC)
                ]
                # u-outer / h-inner: adjacent matmuls hit disjoint PE row
                # groups (partitions 0-63 vs 64-127) and run concurrently
                for u in range(QC // MM_N):
                    for h in range(HPC):
                        hs = slice(h * HD, (h + 1) * HD)
                        nc.tensor.matmul(
                            sts[h][:, bass.ts(u, MM_N)],
                            kt2_sb[hs, bass.ts(t, KT)],
                            qt_sb[hs, jc * QC + u * MM_N:
                                  jc * QC + (u + 1) * MM_N],
                            start=True, stop=True,
                        )
                # softmax numerators: split between the Act engine (exact
                # spline exp) and the DVE (bf16 Schraudolph: exp(x) ~=
                # bitcast_bf16(int16(A*x + B)), one tensor_scalar per tile)
                # so the two engines exponentiate concurrently. Chunk-boundary
                # tiles go to Act: the DVE is busy there with normalize and
                # the output-projection casts.
                pts = []
                for h in range(HPC):
                    if 2 <= t < 30 and (t * HPC + h) % 7 in (1, 4, 6):
                        pt = pt_pool.tile(
                            [128, QC], mybir.dt.int16, tag="pt", name="pta")
                        nc.vector.tensor_scalar(
                            pt, sts[h], SCH_A, SCH_B,
                            mybir.AluOpType.mult, mybir.AluOpType.add,
                        )
                    else:
                        pt = pt_pool.tile([128, QC], bf16, tag="pt")
                        nc.scalar.activation(
                            pt, sts[h], mybir.ActivationFunctionType.Exp,
                            scale=1.0 / 8.0,
                        )
                    pts.append(pt)
                return pts

            def emit_ot_clear(ot_tile):
                # A start=True matmul clears the has_written bits of its
                # whole PSUM bank; the 16 (h, qtile) accumulators are slot-
                # packed 6-per-bank, so clear each bank once via a dummy
                # matmul into the bank's padding column, then run every
                # attnV matmul with start=False: per-element has_written
                # gives overwrite-on-first-touch, accumulate-after.
                for bank in range(3):
                    nc.tensor.matmul(
                        ot_tile[:, bank, MM_N - 1:MM_N],
                        ident, ident[:, 0:1],
                        start=True, stop=True,
                    )

            def emit_attnv(ot_tile, t, pts):
                for h in range(HPC):
                    for qt in range(QC // 128):
                        lhsT = pts[h][:, bass.ts(qt, 128)]
                        if pts[h].dtype == mybir.dt.int16:
                            lhsT = lhsT.bitcast(bf16)
                        nc.tensor.matmul(
                            ot_ap(ot_tile, h, qt),
                            lhsT,
                            v_sb[:, t, :],
                            start=False, stop=(t == NKT - 1),
                            skip_group_check=True,
                        )

            def emit_normalize(jc, ot_tile):
                # bank-ordered so the next chunk's attnV (PE writes, serialized
                # against these DVE reads per PSUM bank) can start on bank 0
                # while banks 1-2 are still being normalized. DVE banks use
                # ONE broadcast tensor_mul per contiguous (h, qt) run; bank 1
                # goes to the Act engine (per-partition scale AP), which idles
                # across chunk boundaries anyway.
                cnts = (6, 6, 4)
                # (bank, slot0, n, h, qt0) contiguous runs of the slot->otn map
                runs = {0: [(0, 6, 0, 0)], 1: [(0, 2, 0, 6), (2, 4, 1, 0)],
                        2: [(0, 4, 1, 4)]}
                for b in range(3):
                    cnt = cnts[b]
                    sl = ot_tile[:, b, 0:cnt * (HD + 1)].rearrange(
                        "p (s e) -> p s e", e=HD + 1)
                    rs = bc_pool.tile([128, 8], fp32, tag="rs")
                    nc.vector.reciprocal(rs[:, 0:cnt], sl[:, :, HD])
                    for (s0, n, h, qt0) in runs[b]:
                        # all on DVE: one broadcast mul per run, keeping the
                        # Act FIFO clear for the next chunk's first exps
                        dst = otn_sb[:, jc * (QC // 128) + qt0:
                                     jc * (QC // 128) + qt0 + n,
                                     h * HD:(h + 1) * HD]
                        rsb = rs[:, s0:s0 + n].rearrange(
                            "p (n o) -> p n o", o=1
                        ).broadcast_to([128, n, HD])
                        nc.vector.tensor_mul(
                            dst, sl[:, s0:s0 + n, 0:HD], rsb)

            def emit_outproj(gqt, pool, tag):
                # transpose otn qtile back to (attn-dims, q) and project
                tp = pool.tile([128, 128], bf16, tag=tag, name="tp")
                nc.tensor.transpose(tp, otn_sb[:, gqt, :], ident)
                ott = ott_pool.tile([128, 128], bf16, tag="ott")
                nc.vector.tensor_copy(ott, tp)
                for u in range(D // MM_N):
                    yp = pool.tile([128, MM_N], fp32, tag=tag, name="yp")
                    nc.tensor.matmul(
                        yp, ott, ow_sb[:, bass.ts(u, MM_N)],
                        start=True, stop=True,
                    )
                    ysb = ysb_pool.tile([128, MM_N], bf16, tag="ysb")
                    nc.vector.tensor_copy(ysb, yp)
                    nc.sync.dma_start(
                        out=y[:, :][bass.ts(gqt, 128), bass.ts(u, MM_N)],
                        in_=ysb,
                    )

            # ---- prologue: minimum projections to start attention ----
            # warm-up burst: ~40 identity matmuls keep the PE busy through
            # the HAM activity window while the xt DMA lands, so the first
            # projection matmuls run at 2.4 GHz instead of 1.2
            wm = ps_aux.tile([128, 128], fp32, tag="aux", name="wm")
            for _ in range(40):
                nc.tensor.matmul(wm, ident, ident, start=True, stop=True)
            # scores t<4 need kv chunk 0; jc0 needs q chunks 0-1. Chunks go
            # through the (still idle) 2-slot scores ring so their matmuls
            # stream back-to-back and warm up the PE clock gate early.
            for ops in (
                proj_chunk_ops(0, "kv", ps_st, "st"),
                proj_chunk_ops(0, "q", ps_st, "st"),
                proj_chunk_ops(1, "q", ps_st, "st"),
            ):
                for op in ops:
                    op()
            for t in range(4):
                emit_vtr(t)

            # the rest of the projection work is drip-fed between attention
            # iterations (aux pool, one live chunk at a time, order matters:
            # kv chunk c feeds score k-tiles [4c, 4c+4))
            pending = []
            for c in range(1, NJ):
                pending.extend(proj_chunk_ops(c, "kv", ps_aux, "aux"))
                for tt in range(4 * c, 4 * c + 4):
                    pending.append(lambda tt=tt: emit_vtr(tt))
            for j in range(2, NJ):
                pending.extend(proj_chunk_ops(j, "q", ps_aux, "aux"))
            pending = pending[::-1]  # pop() from the front

            # ---- attention: flat (jc, t) loop, attnV skewed TWO steps so
            # the PE stream always has ready attnV matmuls queued behind the
            # scores (hides matmul drains; the softmax of t-2 is long done),
            # including across q-chunk boundaries ----
            ot_tiles = {}
            hist = []

            def emit_prev():
                pjc, pt_, ppts = hist.pop(0)
                if pt_ == 0:
                    # alloc + clear right before the chunk's first attnV:
                    # the clear dummies wait on the previous chunk's
                    # normalize reads, so they must sit late in the
                    # strict-FIFO PE stream
                    ot_tiles[pjc] = ps_ot.tile(
                        [128, 3, MM_N], fp32, tag="ot", name="ot_tile")
                    emit_ot_clear(ot_tiles[pjc])
                emit_attnv(ot_tiles[pjc], pt_, ppts)
                if pt_ == NKT - 1:
                    emit_normalize(pjc, ot_tiles.pop(pjc))

            for jc in range(NQC):
                for t in range(NKT):
                    pts = emit_scores_exp(jc, t)
                    hist.append((jc, t, pts))
                    if len(hist) > 2:
                        emit_prev()
                    # background work AFTER scores+softmax+attnV: its DVE
                    # pieces (casts/copies) queue behind the latency-critical
                    # Schraudolph in the strict-FIFO DVE stream, and its PE
                    # pieces deepen the PE queue instead of starving it
                    if jc == 0:
                        for _ in range(5):
                            if pending:
                                pending.pop()()
                    elif t % 4 == 3:
                        emit_outproj(
                            (jc - 1) * (QC // 128) + (t - 3) // 4,
                            ps_aux, "aux",
                        )
                    elif t % 4 == 1:
                        for _ in range(4):
                            if pending:
                                pending.pop()()
            while hist:
                emit_prev()
            # epilogue: last q-chunk's output projection through the now-idle
            # scores PSUM ring, phase-split (all transposes, then all matmuls)
            # so each stage pipelines across the ring slots
            otts = []
            for qt in range(QC // 128):
                gqt = (NQC - 1) * (QC // 128) + qt
                tp = ps_st.tile([128, 128], bf16, tag="st", name="tp")
                nc.tensor.transpose(tp, otn_sb[:, gqt, :], ident)
                ott = ott_pool.tile([128, 128], bf16, tag="ott")
                nc.vector.tensor_copy(ott, tp)
                otts.append((gqt, ott))
            for gqt, ott in otts:
                for u in range(D // MM_N):
                    yp = ps_st.tile([128, MM_N], fp32, tag="st", name="yp")
                    nc.tensor.matmul(
                        yp, ott, ow_sb[:, bass.ts(u, MM_N)],
                        start=True, stop=True,
                    )
                    ysb = ysb_pool.tile([128, MM_N], bf16, tag="ysb")
                    # Act engine is idle once the exps are done; keep the
                    # epilogue casts off the DVE so pieces pipeline
                    nc.scalar.copy(ysb, yp)
                    nc.sync.dma_start(
                        out=y[:, :][bass.ts(gqt, 128), bass.ts(u, MM_N)],
                        in_=ysb,
                    )

    _split_multi_waits(nc)
    return nc


def _split_multi_waits(nc):
    """This toolchain's walrus accepts at most one sync-wait per datapath
    instruction; move extra waits onto same-engine NoOps placed just before."""
    k = 0
    for f in nc.m.functions:
        for blk in f.blocks:
            out = []
            for inst in blk.instructions:
                si = getattr(inst, "sync_info", None)
                ow_ = list(si.on_wait) if (si and si.on_wait) else []
                if len(ow_) > 1:
                    for w in ow_[:-1]:
                        k += 1
                        nop = bass_rust.InstNoOp(
                            name=f"I-wsplit-{k}", ins=[], outs=[]
                        )
                        nop.engine = inst.engine
                        nop.sync_info = mybir.SyncInfo(
                            on_wait=[w], on_update=[]
                        )
                        out.append(nop)
                    inst.sync_info = mybir.SyncInfo(
                        on_wait=[ow_[-1]], on_update=list(si.on_update or [])
                    )
                out.append(inst)
            blk.instructions = out


def _prep_inputs(X, q_w, q_b, k_w, k_b, v_w, v_b, o_w):
    Xt = np.ascontiguousarray(X.reshape(S, D).T).astype(BF16)
    in_maps = []
    for c in range(NCORES):
        kv = c // (NCORES // KVH)
        qs = slice(c * DQ, (c + 1) * DQ)
        ks = slice(kv * HD, (kv + 1) * HD)
        in_maps.append({
            "xt": Xt,
            "qw": np.ascontiguousarray(q_w[:, qs]).astype(BF16),
            "kvw": np.ascontiguousarray(
                np.concatenate([k_w[:, ks], v_w[:, ks]], axis=1)).astype(BF16),
            "ow": np.ascontiguousarray(o_w[qs, :]).astype(BF16),
            "qb": np.ascontiguousarray(q_b[qs]).reshape(1, DQ).astype(BF16),
            "kvb": np.ascontiguousarray(
                np.concatenate([k_b[ks], v_b[ks]])).reshape(1, DKV).astype(BF16),
        })
    return in_maps


def kernel(X, q_w, q_b, k_w, k_b, v_w, v_b, o_w, o_b, **run_kwargs):
    global _COMPILED
    if _COMPILED is None:
        _COMPILED = build_bass()
    in_maps = _prep_inputs(X, q_w, q_b, k_w, k_b, v_w, v_b, o_w)
    res = run_bass_kernel_spmd(
        _COMPILED, in_maps, list(range(NCORES)), **run_kwargs
    )
    out = np.zeros((S, D), dtype=np.float32)
    for r in res.results:
        out += r["y"].astype(np.float32)
    out = out + o_b.astype(np.float32)[None, :]
    if run_kwargs:
        return out.reshape(B, S, D), res
    return out.reshape(B, S, D)
